# revision 14
# baseline (speedup 1.0000x reference)
"""Multi-head attention on 8 Trainium2 NeuronCores.

Problem shape: x[4, 2048, 1024], H=16 heads, Dh=64, fp32.
Sharding: core c handles batch b = c//2 and heads 8*(c%2) .. 8*(c%2)+8.
Each core computes its 8 heads' attention + the partial W_O contraction
for its batch; the host sums the two half-head partials per batch and
adds b_O (plus the b_V @ W_O constant row, folded host-side since
softmax rows sum to 1).  No collectives needed.

All matmuls run in float32r (fp32 storage, PE rounds to 12-bit
mantissa, 4x the fp32 rate at free-dim >= 256).  Host pre-rounds the
DRAM inputs to fp32r (RNE at 12 low mantissa bits) so DMA-loaded
operands satisfy the verifier's "rounded to FP32r" rule; on-chip
producers (ACT/DVE evictions) write float32r-typed tiles.

Device-side layout (per core, all host-pre-transposed so the kernel
never transposes anything):
  xT   [1024, 2048]  = x[b].T                                 [d, t]
  wqT/wkT/wvT [1024, 512] = W[heads].reshape(512,1024).T      [d, (h,k)]
  woT  [512, 1024]   = W_O[heads].transpose(0,2,1).reshape    [(h,k), d]
  bq/bk [128, 4]     per-partition bias layout (col m = (h,k) m*128..)
Pipeline per core:
  Q^T,K^T = W^T x^T  (+bias via ACT eviction)      [(h,k), t]
  V       = x W^T    ([t, 8*(64+1)] with a ones column per head)
  per head pair, per q-chunk: scores^T = K_h Q_h^T  (row-packed K=64
  pairs), exp on ACT (scale=1/8; scores are O(0.2), no max needed),
  O^T_unnorm/denom = V_aug^T exp^T  ([65, q], denom = row 64),
  normalize via reciprocal + K=1 broadcast matmul + DVE multiply,
  spill O^T to DRAM; finally out = O^T^T woT re-loaded per t-tile.
Output: out [2048, 1024] partial (pre-bias) for this core's batch.
"""

import numpy as np
from contextlib import ExitStack

import concourse.bass as bass
import concourse.mybir as mybir
import concourse.tile as tile
from concourse import bacc
from concourse.bass_utils import run_bass_kernel_spmd

F32 = mybir.dt.float32
F32R = mybir.dt.float32r
AF = mybir.ActivationFunctionType

T = 2048          # tokens
D = 1024          # d_model
HK = 512          # 8 local heads x 64
NH = 8            # local heads
DH = 64           # head dim
NDT = 8           # d-tiles of 128
NTT = 16          # t-tiles of 128
NMT = 4           # (h,k) m-tiles of 128
NQC = 4           # q-chunks of 512
NST = 16          # s-tiles of 128
VW = NH * (DH + 1)  # V_aug width: 8 heads x (64 + ones col)


def build():
    nc = bacc.Bacc("TRN2", target_bir_lowering=False, debug=False)

    xT_d = nc.dram_tensor("xT", [D, T], F32R, kind="ExternalInput").ap()
    wq_d = nc.dram_tensor("wqT", [D, HK], F32R, kind="ExternalInput").ap()
    wk_d = nc.dram_tensor("wkT", [D, HK], F32R, kind="ExternalInput").ap()
    wv_d = nc.dram_tensor("wvT", [D, HK], F32R, kind="ExternalInput").ap()
    wo_d = nc.dram_tensor("woT", [HK, D], F32R, kind="ExternalInput").ap()
    bq_d = nc.dram_tensor("bq", [128, 4], F32, kind="ExternalInput").ap()
    bk_d = nc.dram_tensor("bk", [128, 4], F32, kind="ExternalInput").ap()
    ones_d = nc.dram_tensor("ones", [128, DH], F32R, kind="ExternalInput").ap()
    out_d = nc.dram_tensor("out", [T, D], F32, kind="ExternalOutput").ap()

    with tile.TileContext(nc) as tc, ExitStack() as ctx:
        const = ctx.enter_context(tc.tile_pool(name="const", bufs=1))
        bq_sb = const.tile([128, 4], F32, tag="bq", name="bq")
        bk_sb = const.tile([128, 4], F32, tag="bk", name="bk")
        ones_sb = const.tile([128, DH], F32R, tag="ones", name="ones")
        nc.sync.dma_start(bq_sb[:], bq_d)
        nc.sync.dma_start(bk_sb[:], bk_d)
        nc.sync.dma_start(ones_sb[:], ones_d)

        persist = ctx.enter_context(tc.tile_pool(name="persist", bufs=1))
        KT = [persist.tile([128, T], F32R, tag=f"kt{m}", name=f"kt{m}")
              for m in range(NMT)]
        V = [persist.tile([128, VW], F32R, tag=f"v{t}", name=f"v{t}")
             for t in range(NTT)]

        xpool = ctx.enter_context(tc.tile_pool(name="xpool", bufs=2))
        wqpool = ctx.enter_context(tc.tile_pool(name="wqpool", bufs=1))
        wq_sb = [wqpool.tile([128, HK], F32R, tag=f"wq{i}", name=f"wq{i}")
                 for i in range(NDT)]

        # ---------------- phase A: K^T and V projections ----------------
        with tc.tile_pool(name="wkv", bufs=1) as wkv, \
             tc.tile_pool(name="qkv_ps", bufs=4, space="PSUM") as qps:
            wk_sb = [wkv.tile([128, HK], F32R, tag=f"wk{i}", name=f"wk{i}")
                     for i in range(NDT)]
            wv_sb = [wkv.tile([128, HK], F32R, tag=f"wv{i}", name=f"wv{i}")
                     for i in range(NDT)]
            for i in range(NDT):
                nc.sync.dma_start(wk_sb[i][:], wk_d[i * 128:(i + 1) * 128, :])
                nc.sync.dma_start(wv_sb[i][:], wv_d[i * 128:(i + 1) * 128, :])
            for i in range(NDT):
                nc.sync.dma_start(wq_sb[i][:], wq_d[i * 128:(i + 1) * 128, :])
            for c in range(4):  # t-chunks of 512
                csl = slice(c * 512, (c + 1) * 512)
                xt = [xpool.tile([128, 512], F32R, tag=f"x{i}", name=f"x{i}")
                      for i in range(NDT)]
                for i in range(NDT):
                    nc.sync.dma_start(xt[i][:], xT_d[i * 128:(i + 1) * 128, csl])
                for m in range(NMT):
                    msl = slice(m * 128, (m + 1) * 128)
                    ps = qps.tile([128, 512], F32, tag="ps", name="ps")
                    for i in range(NDT):
                        nc.tensor.matmul(ps[:], wk_sb[i][:, msl], xt[i][:],
                                         start=(i == 0), stop=(i == NDT - 1))
                    nc.vector.tensor_scalar_add(KT[m][:, csl], ps[:],
                                                bk_sb[:, m:m + 1])
                for vt in range(4):
                    t_idx = c * 4 + vt
                    vsl = slice(vt * 128, (vt + 1) * 128)
                    ps = qps.tile([128, 512], F32, tag="ps", name="ps")
                    for i in range(NDT):
                        nc.tensor.matmul(ps[:], xt[i][:, vsl], wv_sb[i][:],
                                         start=(i == 0), stop=(i == NDT - 1))
                    v3 = V[t_idx][:].rearrange("p (h c) -> p h c", c=DH + 1)
                    nc.vector.tensor_copy(
                        v3[:, :, 0:DH], ps[:].rearrange("p (h c) -> p h c", c=DH))
                    nc.vector.tensor_copy(
                        v3[:, :, DH:DH + 1],
                        ones_sb[:, 0:NH].rearrange("p (h o) -> p h o", o=1))

        # -------- phase B: per q-chunk: Q^T burst, attention, projection ----
        # The dense 32-matmul Q^T burst before each chunk keeps the PE's
        # HAM clock-gate warm across the ACT-bound attention stretches.
        # scores^T for (st,st+1) land in one 2-bank psum tile so exp runs
        # at N=1024; the two heads of a pair are emitted adjacently so
        # their K=64 matmuls run concurrently on separate PE row groups.
        # AV consumes each exp tile right away, accumulating O^T/denom in
        # a [65, 512] psum per head, copied to SBUF immediately so the
        # slot frees for the next chunk.  The softmax reciprocal runs in a
        # [128, 4] reshape (DVE reciprocal is serial per lane), the row is
        # replicated across partitions by GpSimd partition_broadcast, and
        # multiplied in on DVE.  The output projection follows per chunk.
        with tc.tile_pool(name="qtpool", bufs=2) as qtpool, \
             tc.tile_pool(name="epool", bufs=1) as epool, \
             tc.tile_pool(name="otpool", bufs=1) as otpool, \
             tc.tile_pool(name="fwp", bufs=1) as fwp, \
             tc.tile_pool(name="sc_ps", bufs=2, space="PSUM") as scps, \
             tc.tile_pool(name="av_ps", bufs=1, space="PSUM") as avps, \
             tc.tile_pool(name="fps", bufs=2, space="PSUM") as fps, \
             tc.tile_pool(name="opool", bufs=2) as opool, \
             tc.tile_pool(name="foutp", bufs=2) as foutp:
            wo_sb = [fwp.tile([128, D], F32R, tag=f"wo{jj}", name=f"wo{jj}")
                     for jj in range(NMT)]
            for jj in range(NMT):
                nc.sync.dma_start(wo_sb[jj][:], wo_d[jj * 128:(jj + 1) * 128, :])
            for qc in range(NQC):
                qsl = slice(qc * 512, (qc + 1) * 512)
                # Q^T for this chunk only, [128, 512] per m-tile
                xt = [xpool.tile([128, 512], F32R, tag=f"x{i}", name=f"x{i}")
                      for i in range(NDT)]
                for i in range(NDT):
                    nc.sync.dma_start(xt[i][:], xT_d[i * 128:(i + 1) * 128, qsl])
                QT = [qtpool.tile([128, 512], F32R, tag=f"qt{m}", name=f"qt{m}")
                      for m in range(NMT)]
                for m in range(NMT):
                    msl = slice(m * 128, (m + 1) * 128)
                    ps = fps.tile([128, 512], F32, tag="fp", name="qtp")
                    for i in range(NDT):
                        nc.tensor.matmul(ps[:], wq_sb[i][:, msl], xt[i][:],
                                         start=(i == 0), stop=(i == NDT - 1))
                    nc.vector.tensor_scalar_add(QT[m][:], ps[:],
                                                bq_sb[:, m:m + 1])
                OT = [otpool.tile([128, 512], F32R, tag=f"ot{j}", name=f"ot{j}")
                      for j in range(NMT)]
                for j in range(NMT):  # head pair j: heads 2j, 2j+1
                    avp = {}
                    for hl in (0, 1):
                        avp[hl] = avps.tile([DH + 1, 512], F32,
                                            tag=f"av{hl}", name=f"av{hl}")
                    for sp in range(NST // 2):  # s-tile pairs
                        sc = {}
                        for hl in (0, 1):
                            sc[hl] = scps.tile([128, 1024], F32,
                                               tag="sc", name="sc")
                        for k in (0, 1):
                            st = 2 * sp + k
                            ssl = slice(st * 128, (st + 1) * 128)
                            for hl in (0, 1):
                                psl = slice(hl * 64, (hl + 1) * 64)
                                nc.tensor.matmul(
                                    sc[hl][:, k * 512:(k + 1) * 512],
                                    KT[j][psl, ssl], QT[j][psl, :])
                        es = {}
                        for hl in (0, 1):
                            e = epool.tile([128, 1024], F32R,
                                           tag=f"e{hl}_{sp % 3}",
                                           name=f"e{hl}_{sp % 3}")
                            nc.scalar.activation(e[:], sc[hl][:], AF.Exp,
                                                 scale=0.125)
                            es[hl] = e
                        for hl in (0, 1):
                            h = 2 * j + hl
                            for k in (0, 1):
                                st = 2 * sp + k
                                nc.tensor.matmul(
                                    avp[hl][:],
                                    V[st][:, h * 65:h * 65 + 65],
                                    es[hl][:, k * 512:(k + 1) * 512],
                                    start=(st == 0), stop=(st == NST - 1))
                    for hl in (0, 1):
                        # copy accumulator out of PSUM right away so the
                        # slot frees for the next chunk's AV matmuls
                        avs = opool.tile([DH + 1, 512], F32, tag="avs",
                                         name="avs")
                        nc.vector.tensor_copy(avs[:], avp[hl][:])
                        dn4 = opool.tile([128, 4], F32, tag="dn4", name="dn4")
                        nc.sync.dma_start(dn4[:], avs[DH:DH + 1, :])
                        rc4 = opool.tile([128, 4], F32R, tag="rc4", name="rc4")
                        with nc.allow_low_precision(reason="fp32r softmax recip"):
                            nc.vector.reciprocal(rc4[:], dn4[:])
                        rcp = opool.tile([1, 512], F32R, tag="rcp", name="rcp", bufs=1)
                        nc.sync.dma_start(rcp[:], rc4[:])
                        bcs = opool.tile([DH, 512], F32R, tag="bcs", name="bcs")
                        nc.gpsimd.partition_broadcast(bcs[:], rcp[:])
                        nc.vector.tensor_mul(OT[j][hl * 64:(hl + 1) * 64, :],
                                             avs[0:DH, :], bcs[:])
                # ---- output projection for this q-chunk ----
                for tt in range(4):
                    tq = qc * 512 + tt * 128
                    for dc in range(2):
                        dsl = slice(dc * 512, (dc + 1) * 512)
                        ps = fps.tile([128, 512], F32, tag="fp", name="fp")
                        for jj in range(NMT):
                            nc.tensor.matmul(ps[:],
                                             OT[jj][:, tt * 128:(tt + 1) * 128],
                                             wo_sb[jj][:, dsl],
                                             start=(jj == 0),
                                             stop=(jj == NMT - 1))
                        ob = foutp.tile([128, 512], F32, tag="ob", name="ob")
                        nc.vector.tensor_copy(ob[:], ps[:])
                        nc.sync.dma_start(out_d[tq:tq + 128, dsl], ob[:])

    nc.compile()
    return nc


_NC_CACHE = None


def _get_nc():
    global _NC_CACHE
    if _NC_CACHE is None:
        _NC_CACHE = build()
    return _NC_CACHE


def _round_f32r(x):
    b = np.ascontiguousarray(x, dtype=np.float32).view(np.uint32)
    r = (b + 0x7FF + ((b >> 12) & 1)) & np.uint32(0xFFFFF000)
    return r.view(np.float32)


def _prep_core(x, W_Q, b_Q, W_K, b_K, W_V, b_V, W_O, core):
    b = core // 2
    hs = slice(8 * (core % 2), 8 * (core % 2) + 8)
    f32 = np.float32

    def bias_layout(bx):
        return np.ascontiguousarray(bx[hs].reshape(4, 128).T, dtype=f32)

    return {
        "xT": _round_f32r(x[b].T),
        "wqT": _round_f32r(W_Q[hs].reshape(HK, D).T),
        "wkT": _round_f32r(W_K[hs].reshape(HK, D).T),
        "wvT": _round_f32r(W_V[hs].reshape(HK, D).T),
        "woT": _round_f32r(W_O[hs].transpose(0, 2, 1).reshape(HK, D)),
        "bq": bias_layout(b_Q),
        "bk": bias_layout(b_K),
        "ones": np.ones((128, DH), dtype=f32),
    }


def kernel(x, W_Q, b_Q, W_K, b_K, W_V, b_V, W_O, b_O, _trace=False):
    nc = _get_nc()
    in_maps = [
        _prep_core(x, W_Q, b_Q, W_K, b_K, W_V, b_V, W_O, c) for c in range(8)
    ]
    res = run_bass_kernel_spmd(nc, in_maps, core_ids=list(range(8)),
                               trace=_trace)
    out = np.empty((4, T, D), dtype=np.float32)
    for b in range(4):
        # b_V enters additively after softmax (rows sum to 1): fold
        # b_V @ W_O per half-head shard into the host-side bias.
        acc = res.results[2 * b]["out"].astype(np.float32).copy()
        acc += res.results[2 * b + 1]["out"]
        bias = b_O.astype(np.float64).copy()
        for c in (2 * b, 2 * b + 1):
            hs = slice(8 * (c % 2), 8 * (c % 2) + 8)
            bias += np.einsum("hk,hdk->d", b_V[hs].astype(np.float64),
                              W_O[hs].astype(np.float64))
        out[b] = acc + bias.astype(np.float32)[None, :]
    if _trace:
        kernel.last_results = res
    return out
